# revision 32
# baseline (speedup 1.0000x reference)
"""Discounted cumsum (y[b,h,t,d] = x[b,h,t,d] + gamma[h] * y[b,h,t-1,d]) on 8 trn2 cores.

Pure data parallelism: 64 (b,h) pairs, 8 per core. Two per-pair execution paths
split the work across engines (the DVE scan instruction runs at a fixed ~2
cycles/elem, so the Vector engine alone would cap at ~69 us/core; PE matmuls
run ~1.1 ns/column under the hardware power throttle, so a PE pair costs ~2x
a DVE pair - NV=5/NP=3 balances the two pipelines):

- NV pairs on the Vector engine: layout [d=128 partitions, s=4096 free]; the
  whole recurrence is ONE tensor_tensor_scan (state = gamma*state + x, fp32
  internal state). gamma stays EXACT fp32 via a stride-0 broadcast AP - a
  rounded gamma would be raised to the power t by the recurrence, amplifying
  its error by ~t, while independently-rounded coefficient TABLES (PE path)
  and fp16 data are fine at the 2e-2 gate.

- NP pairs on the Tensor engine (blocked parallel scan, single fp16): layout
  [t-within-block=128 partitions, 32 blocks x 128 d free]. Block sums r_k
  from u=gamma^(127-s) matmuls, carries C_k from one gamma^128-power matmul,
  carry injection as K=1 matmuls of gamma^(t+1) x C accumulated onto the
  within-block scan matmuls A[s,t]=gamma^(t-s) in PSUM. The scan matmul goes
  FIRST in each PSUM bank (start=True resets the whole bank region - a start
  on a sub-slice wipes earlier accumulation).

Engine streams execute in program order and a DMA instruction's semaphore
wait stalls both its engine and its ring FIFO, so phases are emitted in an
order that keeps every stream stall-free:
- sync ring: x loads interleaved with yv outs (each out becomes ready just
  before the next load is needed - self-pacing).
- scalar ring: block-sum scatter + carry gather transposes (enqueued the
  moment their data is ready, so the GT/inject matmuls are never blocked on
  the slow gpsimd software ring) and the yp outs.
- gpsimd ring: constants only.
- PE stream: pair q+1 block sums are emitted before pair q's carry-dependent
  half, so the in-order tensor stream always has work while a carry resolves.

I/O is fp16 both ways (absmax error ~1e-2 against an absolute budget of 0.2);
all transposes are host-side so every HBM DMA is contiguous 8KB lines.
"""

import numpy as np

B, H, S, D = 4, 16, 4096, 128
T = 128          # block length (PE path matmul contraction dim)
KB = S // T      # 32 blocks
NG = 4           # blocks per matmul group (4*128 = 512 moving columns)
G = KB // NG     # 8 groups per pair
NCORES = 8
PAIRS = (B * H) // NCORES  # 8 pair-slots per core
NV = 5           # pairs on the Vector scan path
NP = PAIRS - NV  # pairs on the Tensor (PE) path

_nc_cache = {}


def _build_program():
    if "nc" in _nc_cache:
        return _nc_cache["nc"]

    import concourse.bass as bass
    import concourse.mybir as mybir
    from concourse.tile import TileContext

    f32 = mybir.dt.float32
    fp16 = mybir.dt.float16

    nc = bass.Bass(trn_type="TRN2")

    xv_d = nc.declare_dram_parameter("xv", [NV, D, S], fp16, isOutput=False)
    g_d = nc.declare_dram_parameter("g", [D, NV], f32, isOutput=False)
    yv_d = nc.declare_dram_parameter("yv", [NV, D, S], fp16, isOutput=True)

    xp_d = nc.declare_dram_parameter("xp", [NP, T, KB * D], fp16, isOutput=False)
    A_d = nc.declare_dram_parameter("A_all", [T, NP * T], fp16, isOutput=False)
    u_d = nc.declare_dram_parameter("u_all", [T, NP], fp16, isOutput=False)
    gv_d = nc.declare_dram_parameter("gv_all", [1, NP * T], fp16, isOutput=False)
    GT_d = nc.declare_dram_parameter("GT_all", [KB, NP * KB], fp16, isOutput=False)
    yp_d = nc.declare_dram_parameter("yp", [NP, T, KB * D], fp16, isOutput=True)

    mult, add = mybir.AluOpType.mult, mybir.AluOpType.add

    with TileContext(nc) as tc:
        with (
            tc.tile_pool(name="const", bufs=1) as cpool,
            tc.tile_pool(name="xvin", bufs=3) as xvpool,
            tc.tile_pool(name="yvout", bufs=3) as yvpool,
            tc.tile_pool(name="xpin", bufs=3) as xppool,
            tc.tile_pool(name="ypout", bufs=2) as yppool,
            tc.tile_pool(name="rfl", bufs=2) as rfpool,
            tc.tile_pool(name="r32", bufs=2) as r32pool,
            tc.tile_pool(name="c32", bufs=2) as c32pool,
            tc.tile_pool(name="cfl", bufs=2) as cfpool,
            tc.tile_pool(name="grp_ps", bufs=2, space="PSUM") as gp_pool,
            tc.tile_pool(name="sum_ps", bufs=2, space="PSUM") as sp_pool,
            tc.tile_pool(name="c_ps", bufs=2, space="PSUM") as cp_pool,
        ):
            gc = cpool.tile([D, NV], f32, tag="gc")
            nc.gpsimd.dma_start(out=gc[:], in_=g_d[:])
            Ac = cpool.tile([T, NP * T], fp16, tag="Ac")
            uc = cpool.tile([T, NP], fp16, tag="uc")
            gvc = cpool.tile([1, NP * T], fp16, tag="gvc")
            GTc = cpool.tile([KB, NP * KB], fp16, tag="GTc")
            nc.gpsimd.dma_start(out=Ac[:], in_=A_d[:])
            nc.gpsimd.dma_start(out=uc[:], in_=u_d[:])
            nc.gpsimd.dma_start(out=gvc[:], in_=gv_d[:])
            nc.gpsimd.dma_start(out=GTc[:], in_=GT_d[:])

            v_state = {}
            pe_state = {}

            def v_scan(v):
                X = xvpool.tile([D, S], fp16, tag="Xv")
                nc.sync.dma_start(out=X[:], in_=xv_d[v])
                Y = yvpool.tile([D, S], fp16, tag="Yv")
                nc.vector.tensor_tensor_scan(
                    out=Y[:],
                    data0=gc[:, v : v + 1].broadcast_to([D, S]),
                    data1=X[:],
                    initial=0.0,
                    op0=mult,
                    op1=add,
                )
                v_state[v] = Y

            def v_out(v):
                Y = v_state.pop(v)
                nc.sync.dma_start(out=yv_d[v], in_=Y[:])

            def pe_phase_a(q):
                X = xppool.tile([T, KB * D], fp16, tag="Xp")
                nc.sync.dma_start(out=X[:], in_=xp_d[q])

                # block sums r_k = sum_s gamma^(127-s) x_s  -> [1, (k d)]
                Rflat = rfpool.tile([1, KB * D], fp16, tag="Rflat")
                for g in range(G):
                    sl = slice(g * NG * D, (g + 1) * NG * D)
                    rp = sp_pool.tile([1, NG * D], f32, tag="rp")
                    nc.tensor.matmul(
                        rp[:], lhsT=uc[:, q : q + 1], rhs=X[:, sl],
                        start=True, stop=True,
                    )
                    nc.scalar.copy(out=Rflat[:, sl], in_=rp[:])
                # scatter [1,(k d)] -> [KB part, d]
                R32 = r32pool.tile([KB, D], fp16, tag="R32")
                nc.scalar.dma_start(out=R32[:], in_=Rflat[:])
                pe_state[q] = (X, R32)

            def pe_phase_bgt(q):
                X, R32 = pe_state.pop(q)
                # carries C[k] = sum_{j<k} gamma^(128(k-1-j)) r_j
                cp = cp_pool.tile([KB, D], f32, tag="cp")
                nc.tensor.matmul(
                    cp[:], lhsT=GTc[:, q * KB : (q + 1) * KB], rhs=R32[:],
                    start=True, stop=True,
                )
                C32 = c32pool.tile([KB, D], fp16, tag="C32")
                nc.scalar.copy(out=C32[:], in_=cp[:])
                cfh = cfpool.tile([1, KB * D], fp16, tag="cfh")
                nc.scalar.dma_start(out=cfh[:], in_=C32[:])
                pe_state[q] = (X, cfh)

            def pe_phase_scan(q):
                X, cfh = pe_state.pop(q)
                Ys = yppool.tile([T, KB * D], fp16, tag="Ys")
                Aq = Ac[:, q * T : (q + 1) * T]
                gvq = gvc[:, q * T : (q + 1) * T]
                for g2 in range(G // 2):
                    sl2 = slice(g2 * 2 * NG * D, (g2 + 1) * 2 * NG * D)
                    grp = gp_pool.tile([T, 2 * NG * D], f32, tag="grp")
                    for h in range(2):
                        g = g2 * 2 + h
                        slh = slice(h * NG * D, (h + 1) * NG * D)
                        slx = slice(g * NG * D, (g + 1) * NG * D)
                        nc.tensor.matmul(
                            grp[:, slh], lhsT=Aq, rhs=X[:, slx],
                            start=True, stop=False,
                        )
                        nc.tensor.matmul(
                            grp[:, slh], lhsT=gvq, rhs=cfh[:, slx],
                            start=False, stop=True,
                        )
                    nc.scalar.copy(out=Ys[:, sl2], in_=grp[:])
                    if g2 % 2 == 1:
                        slo = slice((g2 - 1) * 2 * NG * D, (g2 + 1) * 2 * NG * D)
                        nc.scalar.dma_start(out=yp_d[q, :, slo], in_=Ys[:, slo])

            for kind, idx in [
                ("vs", 0), ("a", 0), ("vs", 1), ("a", 1), ("gt", 0),
                ("vo", 0), ("vs", 2), ("sc", 0), ("a", 2), ("gt", 1),
                ("vo", 1), ("vs", 3), ("sc", 1), ("gt", 2), ("vo", 2),
                ("vs", 4), ("sc", 2), ("vo", 3), ("vo", 4),
            ]:
                {
                    "vs": v_scan,
                    "vo": v_out,
                    "a": pe_phase_a,
                    "gt": pe_phase_bgt,
                    "sc": pe_phase_scan,
                }[kind](idx)

    # Walrus allows 1 sync wait on engine instructions / 2 on DMAs; move
    # excess waits onto InstEventSemaphore carriers.
    import bass_rust

    bass_rust.generate_event_semaphores(nc)

    _nc_cache["nc"] = nc
    return nc


def _pe_constants(g):
    """fp16 coefficient tables from float64 gamma powers."""
    pw = np.power(g, np.arange(S, dtype=np.float64))
    t_idx = np.arange(T)
    t_minus_s = t_idx[None, :] - t_idx[:, None]
    A = np.where(t_minus_s >= 0, pw[np.clip(t_minus_s, 0, None)], 0.0)
    u = pw[127 - t_idx]
    gv = pw[t_idx + 1]
    pw128 = np.power(pw[T], np.arange(KB, dtype=np.float64))
    k_minus_j = np.arange(KB)[None, :] - 1 - np.arange(KB)[:, None]
    GT = np.where(k_minus_j >= 0, pw128[np.clip(k_minus_j, 0, None)], 0.0)
    return A, u, gv, GT


def _make_in_maps(tensor, gamma):
    x = np.asarray(tensor, dtype=np.float32).reshape(B * H, S, D)
    gam = np.asarray(gamma, dtype=np.float32).reshape(H)

    in_maps = []
    for c in range(NCORES):
        pids = [c * PAIRS + p for p in range(PAIRS)]
        # vector-path pairs: [D, S] fp16, scan axis last
        xv = np.empty((NV, D, S), np.float16)
        gcol = np.empty((D, NV), np.float32)
        for v in range(NV):
            pid = pids[v]
            xv[v] = x[pid].T.astype(np.float16)
            gcol[:, v] = gam[pid % H]
        # PE-path pairs: scan layout [s-within-block, (block, d)] fp16
        xp = np.empty((NP, T, KB * D), np.float16)
        A_all = np.empty((T, NP * T), np.float16)
        u_all = np.empty((T, NP), np.float16)
        gv_all = np.empty((1, NP * T), np.float16)
        GT_all = np.empty((KB, NP * KB), np.float16)
        for q in range(NP):
            pid = pids[NV + q]
            xp[q] = (
                x[pid].reshape(KB, T, D).transpose(1, 0, 2).reshape(T, KB * D)
                .astype(np.float16)
            )
            A, u, gv, GT = _pe_constants(float(gam[pid % H]))
            A_all[:, q * T : (q + 1) * T] = A.astype(np.float16)
            u_all[:, q] = u.astype(np.float16)
            gv_all[0, q * T : (q + 1) * T] = gv.astype(np.float16)
            GT_all[:, q * KB : (q + 1) * KB] = GT.astype(np.float16)
        in_maps.append(
            {
                "xv": xv,
                "g": gcol,
                "xp": xp,
                "A_all": A_all,
                "u_all": u_all,
                "gv_all": gv_all,
                "GT_all": GT_all,
            }
        )
    return in_maps


def kernel(tensor, gamma):
    from concourse.bass_utils import run_bass_kernel_spmd

    in_maps = _make_in_maps(tensor, gamma)
    nc = _build_program()
    res = run_bass_kernel_spmd(nc, in_maps, list(range(NCORES))).results
    y = np.empty((B * H, S, D), np.float32)
    for c in range(NCORES):
        yv = np.asarray(res[c]["yv"])  # [NV, D, S] fp16
        yp = np.asarray(res[c]["yp"])  # [NP, T, KB*D] fp16
        for v in range(NV):
            y[c * PAIRS + v] = yv[v].T
        for q in range(NP):
            y[c * PAIRS + NV + q] = (
                yp[q].reshape(T, KB, D).transpose(1, 0, 2).reshape(S, D)
            )
    return y.reshape(B, H, S, D)


# revision 34
# speedup vs baseline: 1.0218x; 1.0218x over previous
"""Discounted cumsum (y[b,h,t,d] = x[b,h,t,d] + gamma[h] * y[b,h,t-1,d]) on 8 trn2 cores.

Pure data parallelism: 64 (b,h) pairs, 8 per core. Two per-pair execution paths
split the work across engines (the DVE scan instruction runs at a fixed ~2
cycles/elem, so the Vector engine alone would cap at ~69 us/core; PE matmuls
run ~1.1 ns/column under the hardware power throttle, so a PE pair costs ~2x
a DVE pair - NV=5/NP=3 balances the two pipelines):

- NV pairs on the Vector engine: layout [d=128 partitions, s=4096 free]; the
  whole recurrence is ONE tensor_tensor_scan (state = gamma*state + x, fp32
  internal state). gamma stays EXACT fp32 via a stride-0 broadcast AP - a
  rounded gamma would be raised to the power t by the recurrence, amplifying
  its error by ~t, while independently-rounded coefficient TABLES (PE path)
  and fp16 data are fine at the 2e-2 gate.

- NP pairs on the Tensor engine (blocked parallel scan, single fp16): layout
  [t-within-block=128 partitions, 32 blocks x 128 d free]. Block sums r_k
  from u=gamma^(127-s) matmuls, carries C_k from one gamma^128-power matmul,
  carry injection as K=1 matmuls of gamma^(t+1) x C accumulated onto the
  within-block scan matmuls A[s,t]=gamma^(t-s) in PSUM. The scan matmul goes
  FIRST in each PSUM bank (start=True resets the whole bank region - a start
  on a sub-slice wipes earlier accumulation).

Engine streams execute in program order and a DMA instruction's semaphore
wait stalls both its engine and its ring FIFO, so phases are emitted in an
order that keeps every stream stall-free:
- sync ring: x loads interleaved with yv outs (each out becomes ready just
  before the next load is needed - self-pacing).
- scalar ring: block-sum scatter + carry gather transposes (enqueued the
  moment their data is ready, so the GT/inject matmuls are never blocked on
  the slow gpsimd software ring) and the yp outs.
- gpsimd ring: constants only.
- PE stream: pair q+1 block sums are emitted before pair q's carry-dependent
  half, so the in-order tensor stream always has work while a carry resolves.

I/O is fp16 both ways (absmax error ~1e-2 against an absolute budget of 0.2);
all transposes are host-side so every HBM DMA is contiguous 8KB lines.
"""

import numpy as np

B, H, S, D = 4, 16, 4096, 128
T = 128          # block length (PE path matmul contraction dim)
KB = S // T      # 32 blocks
NG = 4           # blocks per matmul group (4*128 = 512 moving columns)
G = KB // NG     # 8 groups per pair
NCORES = 8
PAIRS = (B * H) // NCORES  # 8 pair-slots per core
NV = 5           # pairs on the Vector scan path
NP = PAIRS - NV  # pairs on the Tensor (PE) path

_nc_cache = {}


def _build_program():
    if "nc" in _nc_cache:
        return _nc_cache["nc"]

    import concourse.bass as bass
    import concourse.mybir as mybir
    from concourse.tile import TileContext

    f32 = mybir.dt.float32
    fp16 = mybir.dt.float16

    nc = bass.Bass(trn_type="TRN2")

    xv_d = nc.declare_dram_parameter("xv", [NV, D, S], fp16, isOutput=False)
    g_d = nc.declare_dram_parameter("g", [D, NV], f32, isOutput=False)
    yv_d = nc.declare_dram_parameter("yv", [NV, D, S], fp16, isOutput=True)

    xp_d = nc.declare_dram_parameter("xp", [NP, T, KB * D], fp16, isOutput=False)
    A_d = nc.declare_dram_parameter("A_all", [T, NP * T], fp16, isOutput=False)
    u_d = nc.declare_dram_parameter("u_all", [T, NP], fp16, isOutput=False)
    gv_d = nc.declare_dram_parameter("gv_all", [1, NP * T], fp16, isOutput=False)
    GT_d = nc.declare_dram_parameter("GT_all", [KB, NP * KB], fp16, isOutput=False)
    yp_d = nc.declare_dram_parameter("yp", [NP, T, KB * D], fp16, isOutput=True)

    mult, add = mybir.AluOpType.mult, mybir.AluOpType.add

    with TileContext(nc) as tc:
        with (
            tc.tile_pool(name="const", bufs=1) as cpool,
            tc.tile_pool(name="xvin", bufs=3) as xvpool,
            tc.tile_pool(name="yvout", bufs=3) as yvpool,
            tc.tile_pool(name="xpin", bufs=3) as xppool,
            tc.tile_pool(name="ypout", bufs=2) as yppool,
            tc.tile_pool(name="rfl", bufs=2) as rfpool,
            tc.tile_pool(name="r32", bufs=2) as r32pool,
            tc.tile_pool(name="c32", bufs=2) as c32pool,
            tc.tile_pool(name="cfl", bufs=2) as cfpool,
            tc.tile_pool(name="grp_ps", bufs=2, space="PSUM") as gp_pool,
            tc.tile_pool(name="sum_ps", bufs=2, space="PSUM") as sp_pool,
            tc.tile_pool(name="c_ps", bufs=2, space="PSUM") as cp_pool,
        ):
            gc = cpool.tile([D, NV], f32, tag="gc")
            nc.gpsimd.dma_start(out=gc[:], in_=g_d[:])
            Ac = cpool.tile([T, NP * T], fp16, tag="Ac")
            uc = cpool.tile([T, NP], fp16, tag="uc")
            gvc = cpool.tile([1, NP * T], fp16, tag="gvc")
            GTc = cpool.tile([KB, NP * KB], fp16, tag="GTc")
            nc.gpsimd.dma_start(out=Ac[:], in_=A_d[:])
            nc.gpsimd.dma_start(out=uc[:], in_=u_d[:])
            nc.gpsimd.dma_start(out=gvc[:], in_=gv_d[:])
            nc.gpsimd.dma_start(out=GTc[:], in_=GT_d[:])

            v_state = {}
            pe_state = {}

            def v_scan(v):
                X = xvpool.tile([D, S], fp16, tag="Xv")
                nc.sync.dma_start(out=X[:], in_=xv_d[v])
                Y = yvpool.tile([D, S], fp16, tag="Yv")
                nc.vector.tensor_tensor_scan(
                    out=Y[:],
                    data0=gc[:, v : v + 1].broadcast_to([D, S]),
                    data1=X[:],
                    initial=0.0,
                    op0=mult,
                    op1=add,
                )
                v_state[v] = Y

            def v_out(v):
                Y = v_state.pop(v)
                nc.scalar.dma_start(out=yv_d[v], in_=Y[:])

            def pe_phase_a(q):
                X = xppool.tile([T, KB * D], fp16, tag="Xp")
                nc.sync.dma_start(out=X[:], in_=xp_d[q])

                # block sums r_k = sum_s gamma^(127-s) x_s  -> [1, (k d)]
                Rflat = rfpool.tile([1, KB * D], fp16, tag="Rflat")
                for g in range(G):
                    sl = slice(g * NG * D, (g + 1) * NG * D)
                    rp = sp_pool.tile([1, NG * D], f32, tag="rp")
                    nc.tensor.matmul(
                        rp[:], lhsT=uc[:, q : q + 1], rhs=X[:, sl],
                        start=True, stop=True,
                    )
                    nc.scalar.copy(out=Rflat[:, sl], in_=rp[:])
                # scatter [1,(k d)] -> [KB part, d]
                R32 = r32pool.tile([KB, D], fp16, tag="R32")
                nc.scalar.dma_start(out=R32[:], in_=Rflat[:])
                pe_state[q] = (X, R32)

            def pe_phase_bgt(q):
                X, R32 = pe_state.pop(q)
                # carries C[k] = sum_{j<k} gamma^(128(k-1-j)) r_j
                cp = cp_pool.tile([KB, D], f32, tag="cp")
                nc.tensor.matmul(
                    cp[:], lhsT=GTc[:, q * KB : (q + 1) * KB], rhs=R32[:],
                    start=True, stop=True,
                )
                C32 = c32pool.tile([KB, D], fp16, tag="C32")
                nc.scalar.copy(out=C32[:], in_=cp[:])
                cfh = cfpool.tile([1, KB * D], fp16, tag="cfh")
                nc.scalar.dma_start(out=cfh[:], in_=C32[:])
                pe_state[q] = (X, cfh)

            def pe_phase_scan(q):
                X, cfh = pe_state.pop(q)
                Ys = yppool.tile([T, KB * D], fp16, tag="Ys")
                Aq = Ac[:, q * T : (q + 1) * T]
                gvq = gvc[:, q * T : (q + 1) * T]
                for g2 in range(G // 2):
                    sl2 = slice(g2 * 2 * NG * D, (g2 + 1) * 2 * NG * D)
                    grp = gp_pool.tile([T, 2 * NG * D], f32, tag="grp")
                    for h in range(2):
                        g = g2 * 2 + h
                        slh = slice(h * NG * D, (h + 1) * NG * D)
                        slx = slice(g * NG * D, (g + 1) * NG * D)
                        nc.tensor.matmul(
                            grp[:, slh], lhsT=Aq, rhs=X[:, slx],
                            start=True, stop=False,
                        )
                        nc.tensor.matmul(
                            grp[:, slh], lhsT=gvq, rhs=cfh[:, slx],
                            start=False, stop=True,
                        )
                    nc.scalar.copy(out=Ys[:, sl2], in_=grp[:])
                    if g2 % 2 == 1:
                        slo = slice((g2 - 1) * 2 * NG * D, (g2 + 1) * 2 * NG * D)
                        nc.scalar.dma_start(out=yp_d[q, :, slo], in_=Ys[:, slo])

            for kind, idx in [
                ("vs", 0), ("a", 0), ("vs", 1), ("a", 1), ("gt", 0),
                ("vo", 0), ("vs", 2), ("sc", 0), ("a", 2), ("gt", 1),
                ("vo", 1), ("vs", 3), ("sc", 1), ("gt", 2), ("vo", 2),
                ("vs", 4), ("vo", 3), ("sc", 2), ("vo", 4),
            ]:
                {
                    "vs": v_scan,
                    "vo": v_out,
                    "a": pe_phase_a,
                    "gt": pe_phase_bgt,
                    "sc": pe_phase_scan,
                }[kind](idx)

    # Walrus allows 1 sync wait on engine instructions / 2 on DMAs; move
    # excess waits onto InstEventSemaphore carriers.
    import bass_rust

    bass_rust.generate_event_semaphores(nc)

    _nc_cache["nc"] = nc
    return nc


def _pe_constants(g):
    """fp16 coefficient tables from float64 gamma powers."""
    pw = np.power(g, np.arange(S, dtype=np.float64))
    t_idx = np.arange(T)
    t_minus_s = t_idx[None, :] - t_idx[:, None]
    A = np.where(t_minus_s >= 0, pw[np.clip(t_minus_s, 0, None)], 0.0)
    u = pw[127 - t_idx]
    gv = pw[t_idx + 1]
    pw128 = np.power(pw[T], np.arange(KB, dtype=np.float64))
    k_minus_j = np.arange(KB)[None, :] - 1 - np.arange(KB)[:, None]
    GT = np.where(k_minus_j >= 0, pw128[np.clip(k_minus_j, 0, None)], 0.0)
    return A, u, gv, GT


def _make_in_maps(tensor, gamma):
    x = np.asarray(tensor, dtype=np.float32).reshape(B * H, S, D)
    gam = np.asarray(gamma, dtype=np.float32).reshape(H)

    in_maps = []
    for c in range(NCORES):
        pids = [c * PAIRS + p for p in range(PAIRS)]
        # vector-path pairs: [D, S] fp16, scan axis last
        xv = np.empty((NV, D, S), np.float16)
        gcol = np.empty((D, NV), np.float32)
        for v in range(NV):
            pid = pids[v]
            xv[v] = x[pid].T.astype(np.float16)
            gcol[:, v] = gam[pid % H]
        # PE-path pairs: scan layout [s-within-block, (block, d)] fp16
        xp = np.empty((NP, T, KB * D), np.float16)
        A_all = np.empty((T, NP * T), np.float16)
        u_all = np.empty((T, NP), np.float16)
        gv_all = np.empty((1, NP * T), np.float16)
        GT_all = np.empty((KB, NP * KB), np.float16)
        for q in range(NP):
            pid = pids[NV + q]
            xp[q] = (
                x[pid].reshape(KB, T, D).transpose(1, 0, 2).reshape(T, KB * D)
                .astype(np.float16)
            )
            A, u, gv, GT = _pe_constants(float(gam[pid % H]))
            A_all[:, q * T : (q + 1) * T] = A.astype(np.float16)
            u_all[:, q] = u.astype(np.float16)
            gv_all[0, q * T : (q + 1) * T] = gv.astype(np.float16)
            GT_all[:, q * KB : (q + 1) * KB] = GT.astype(np.float16)
        in_maps.append(
            {
                "xv": xv,
                "g": gcol,
                "xp": xp,
                "A_all": A_all,
                "u_all": u_all,
                "gv_all": gv_all,
                "GT_all": GT_all,
            }
        )
    return in_maps


def kernel(tensor, gamma):
    from concourse.bass_utils import run_bass_kernel_spmd

    in_maps = _make_in_maps(tensor, gamma)
    nc = _build_program()
    res = run_bass_kernel_spmd(nc, in_maps, list(range(NCORES))).results
    y = np.empty((B * H, S, D), np.float32)
    for c in range(NCORES):
        yv = np.asarray(res[c]["yv"])  # [NV, D, S] fp16
        yp = np.asarray(res[c]["yp"])  # [NP, T, KB*D] fp16
        for v in range(NV):
            y[c * PAIRS + v] = yv[v].T
        for q in range(NP):
            y[c * PAIRS + NV + q] = (
                yp[q].reshape(T, KB, D).transpose(1, 0, 2).reshape(S, D)
            )
    return y.reshape(B, H, S, D)


# revision 37
# speedup vs baseline: 1.2450x; 1.2184x over previous
"""Discounted cumsum (y[b,h,t,d] = x[b,h,t,d] + gamma[h] * y[b,h,t-1,d]) on 8 trn2 cores.

Pure data parallelism: 64 (b,h) pairs, 8 per core. Two per-pair execution paths
split the work across engines (the DVE scan instruction runs at a fixed ~2
cycles/elem, so the Vector engine alone would cap at ~69 us/core; PE matmuls
run ~1.1 ns/column under the hardware power throttle, so a PE pair costs ~2x
a DVE pair - NV=5/NP=3 balances the two pipelines):

- NV pairs on the Vector engine: layout [d=128 partitions, s=4096 free]; the
  whole recurrence is ONE tensor_tensor_scan (state = gamma*state + x, fp32
  internal state). gamma stays EXACT fp32 via a stride-0 broadcast AP - a
  rounded gamma would be raised to the power t by the recurrence, amplifying
  its error by ~t, while independently-rounded coefficient TABLES (PE path)
  and fp16 data are fine at the 2e-2 gate.

- NP pairs on the Tensor engine (blocked parallel scan, single fp16): layout
  [t-within-block=128 partitions, 32 blocks x 128 d free]. Block sums r_k
  from u=gamma^(127-s) matmuls, carries C_k from one gamma^128-power matmul,
  carry injection as K=1 matmuls of gamma^(t+1) x C accumulated onto the
  within-block scan matmuls A[s,t]=gamma^(t-s) in PSUM. The scan matmul goes
  FIRST in each PSUM bank (start=True resets the whole bank region - a start
  on a sub-slice wipes earlier accumulation).

Engine streams execute in program order and a DMA instruction's semaphore
wait stalls both its engine and its ring FIFO, so phases are emitted in an
order that keeps every stream stall-free:
- sync ring: x loads interleaved with yv outs (each out becomes ready just
  before the next load is needed - self-pacing).
- scalar ring: block-sum scatter + carry gather transposes (enqueued the
  moment their data is ready, so the GT/inject matmuls are never blocked on
  the slow gpsimd software ring) and the yp outs.
- gpsimd ring: constants only.
- PE stream: pair q+1 block sums are emitted before pair q's carry-dependent
  half, so the in-order tensor stream always has work while a carry resolves.

I/O is fp16 both ways (absmax error ~1e-2 against an absolute budget of 0.2);
all transposes are host-side so every HBM DMA is contiguous 8KB lines.
"""

import numpy as np

B, H, S, D = 4, 16, 4096, 128
T = 128          # block length (PE path matmul contraction dim)
KB = S // T      # 32 blocks
NG = 4           # blocks per matmul group (4*128 = 512 moving columns)
G = KB // NG     # 8 groups per pair
NCORES = 8
PAIRS = (B * H) // NCORES  # 8 pair-slots per core
NV = 6           # pairs on the Vector scan path
NP = PAIRS - NV  # pairs on the Tensor (PE) path

_nc_cache = {}


def _build_program():
    if "nc" in _nc_cache:
        return _nc_cache["nc"]

    import concourse.bass as bass
    import concourse.mybir as mybir
    from concourse.tile import TileContext

    f32 = mybir.dt.float32
    fp16 = mybir.dt.float16

    nc = bass.Bass(trn_type="TRN2")

    xv_d = nc.declare_dram_parameter("xv", [NV, D, S], fp16, isOutput=False)
    g_d = nc.declare_dram_parameter("g", [D, NV], f32, isOutput=False)
    yv_d = nc.declare_dram_parameter("yv", [NV, D, S], fp16, isOutput=True)

    xp_d = nc.declare_dram_parameter("xp", [NP, T, KB * D], fp16, isOutput=False)
    A_d = nc.declare_dram_parameter("A_all", [T, NP * T], fp16, isOutput=False)
    u_d = nc.declare_dram_parameter("u_all", [T, NP], fp16, isOutput=False)
    gv_d = nc.declare_dram_parameter("gv_all", [1, NP * T], fp16, isOutput=False)
    GT_d = nc.declare_dram_parameter("GT_all", [KB, NP * KB], fp16, isOutput=False)
    yp_d = nc.declare_dram_parameter("yp", [NP, T, KB * D], fp16, isOutput=True)

    mult, add = mybir.AluOpType.mult, mybir.AluOpType.add

    with TileContext(nc) as tc:
        with (
            tc.tile_pool(name="const", bufs=1) as cpool,
            tc.tile_pool(name="xvin", bufs=3) as xvpool,
            tc.tile_pool(name="yvout", bufs=3) as yvpool,
            tc.tile_pool(name="xpin", bufs=3) as xppool,
            tc.tile_pool(name="ypout", bufs=2) as yppool,
            tc.tile_pool(name="rfl", bufs=2) as rfpool,
            tc.tile_pool(name="r32", bufs=2) as r32pool,
            tc.tile_pool(name="c32", bufs=2) as c32pool,
            tc.tile_pool(name="cfl", bufs=2) as cfpool,
            tc.tile_pool(name="grp_ps", bufs=2, space="PSUM") as gp_pool,
            tc.tile_pool(name="sum_ps", bufs=2, space="PSUM") as sp_pool,
            tc.tile_pool(name="c_ps", bufs=2, space="PSUM") as cp_pool,
        ):
            gc = cpool.tile([D, NV], f32, tag="gc")
            nc.gpsimd.dma_start(out=gc[:], in_=g_d[:])
            Ac = cpool.tile([T, NP * T], fp16, tag="Ac")
            uc = cpool.tile([T, NP], fp16, tag="uc")
            gvc = cpool.tile([1, NP * T], fp16, tag="gvc")
            GTc = cpool.tile([KB, NP * KB], fp16, tag="GTc")
            nc.gpsimd.dma_start(out=Ac[:], in_=A_d[:])
            nc.gpsimd.dma_start(out=uc[:], in_=u_d[:])
            nc.gpsimd.dma_start(out=gvc[:], in_=gv_d[:])
            nc.gpsimd.dma_start(out=GTc[:], in_=GT_d[:])

            v_state = {}
            pe_state = {}

            def v_scan(v):
                X = xvpool.tile([D, S], fp16, tag="Xv")
                nc.sync.dma_start(out=X[:], in_=xv_d[v])
                Y = yvpool.tile([D, S], fp16, tag="Yv")
                nc.vector.tensor_tensor_scan(
                    out=Y[:],
                    data0=gc[:, v : v + 1].broadcast_to([D, S]),
                    data1=X[:],
                    initial=0.0,
                    op0=mult,
                    op1=add,
                )
                v_state[v] = Y

            def v_out(v):
                Y = v_state.pop(v)
                nc.scalar.dma_start(out=yv_d[v], in_=Y[:])

            def pe_phase_a(q):
                X = xppool.tile([T, KB * D], fp16, tag="Xp")
                nc.sync.dma_start(out=X[:], in_=xp_d[q])

                # block sums r_k = sum_s gamma^(127-s) x_s  -> [1, (k d)]
                Rflat = rfpool.tile([1, KB * D], fp16, tag="Rflat")
                for g in range(G):
                    sl = slice(g * NG * D, (g + 1) * NG * D)
                    rp = sp_pool.tile([1, NG * D], f32, tag="rp")
                    nc.tensor.matmul(
                        rp[:], lhsT=uc[:, q : q + 1], rhs=X[:, sl],
                        start=True, stop=True,
                    )
                    nc.scalar.copy(out=Rflat[:, sl], in_=rp[:])
                # scatter [1,(k d)] -> [KB part, d]
                R32 = r32pool.tile([KB, D], fp16, tag="R32")
                nc.scalar.dma_start(out=R32[:], in_=Rflat[:])
                pe_state[q] = (X, R32)

            def pe_phase_bgt(q):
                X, R32 = pe_state.pop(q)
                # carries C[k] = sum_{j<k} gamma^(128(k-1-j)) r_j
                cp = cp_pool.tile([KB, D], f32, tag="cp")
                nc.tensor.matmul(
                    cp[:], lhsT=GTc[:, q * KB : (q + 1) * KB], rhs=R32[:],
                    start=True, stop=True,
                )
                C32 = c32pool.tile([KB, D], fp16, tag="C32")
                nc.scalar.copy(out=C32[:], in_=cp[:])
                cfh = cfpool.tile([1, KB * D], fp16, tag="cfh")
                nc.scalar.dma_start(out=cfh[:], in_=C32[:])
                pe_state[q] = (X, cfh)

            def pe_phase_scan(q):
                X, cfh = pe_state.pop(q)
                Ys = yppool.tile([T, KB * D], fp16, tag="Ys")
                Aq = Ac[:, q * T : (q + 1) * T]
                gvq = gvc[:, q * T : (q + 1) * T]
                for g2 in range(G // 2):
                    sl2 = slice(g2 * 2 * NG * D, (g2 + 1) * 2 * NG * D)
                    grp = gp_pool.tile([T, 2 * NG * D], f32, tag="grp")
                    # both scans first (no carry dependency), injects after:
                    # a late cfh then only stalls the injects, not the scans
                    for h in range(2):
                        g = g2 * 2 + h
                        nc.tensor.matmul(
                            grp[:, h * NG * D : (h + 1) * NG * D],
                            lhsT=Aq,
                            rhs=X[:, g * NG * D : (g + 1) * NG * D],
                            start=True, stop=False,
                            skip_group_check=True,
                        )
                    for h in range(2):
                        g = g2 * 2 + h
                        nc.tensor.matmul(
                            grp[:, h * NG * D : (h + 1) * NG * D],
                            lhsT=gvq,
                            rhs=cfh[:, g * NG * D : (g + 1) * NG * D],
                            start=False, stop=True,
                            skip_group_check=True,
                        )
                    nc.scalar.copy(out=Ys[:, sl2], in_=grp[:])
                    if g2 % 2 == 1:
                        slo = slice((g2 - 1) * 2 * NG * D, (g2 + 1) * 2 * NG * D)
                        nc.scalar.dma_start(out=yp_d[q, :, slo], in_=Ys[:, slo])

            for kind, idx in [
                ("vs", 0), ("a", 0), ("vs", 1), ("a", 1), ("gt", 0),
                ("vs", 2), ("vo", 0), ("sc", 0), ("gt", 1), ("vs", 3),
                ("vo", 1), ("sc", 1), ("vs", 4), ("vo", 2), ("vs", 5),
                ("vo", 3), ("vo", 4), ("vo", 5),
            ]:
                {
                    "vs": v_scan,
                    "vo": v_out,
                    "a": pe_phase_a,
                    "gt": pe_phase_bgt,
                    "sc": pe_phase_scan,
                }[kind](idx)

    # Walrus allows 1 sync wait on engine instructions / 2 on DMAs; move
    # excess waits onto InstEventSemaphore carriers.
    import bass_rust

    bass_rust.generate_event_semaphores(nc)

    _nc_cache["nc"] = nc
    return nc


def _pe_constants(g):
    """fp16 coefficient tables from float64 gamma powers."""
    pw = np.power(g, np.arange(S, dtype=np.float64))
    t_idx = np.arange(T)
    t_minus_s = t_idx[None, :] - t_idx[:, None]
    A = np.where(t_minus_s >= 0, pw[np.clip(t_minus_s, 0, None)], 0.0)
    u = pw[127 - t_idx]
    gv = pw[t_idx + 1]
    pw128 = np.power(pw[T], np.arange(KB, dtype=np.float64))
    k_minus_j = np.arange(KB)[None, :] - 1 - np.arange(KB)[:, None]
    GT = np.where(k_minus_j >= 0, pw128[np.clip(k_minus_j, 0, None)], 0.0)
    return A, u, gv, GT


def _make_in_maps(tensor, gamma):
    x = np.asarray(tensor, dtype=np.float32).reshape(B * H, S, D)
    gam = np.asarray(gamma, dtype=np.float32).reshape(H)

    in_maps = []
    for c in range(NCORES):
        pids = [c * PAIRS + p for p in range(PAIRS)]
        # vector-path pairs: [D, S] fp16, scan axis last
        xv = np.empty((NV, D, S), np.float16)
        gcol = np.empty((D, NV), np.float32)
        for v in range(NV):
            pid = pids[v]
            xv[v] = x[pid].T.astype(np.float16)
            gcol[:, v] = gam[pid % H]
        # PE-path pairs: scan layout [s-within-block, (block, d)] fp16
        xp = np.empty((NP, T, KB * D), np.float16)
        A_all = np.empty((T, NP * T), np.float16)
        u_all = np.empty((T, NP), np.float16)
        gv_all = np.empty((1, NP * T), np.float16)
        GT_all = np.empty((KB, NP * KB), np.float16)
        for q in range(NP):
            pid = pids[NV + q]
            xp[q] = (
                x[pid].reshape(KB, T, D).transpose(1, 0, 2).reshape(T, KB * D)
                .astype(np.float16)
            )
            A, u, gv, GT = _pe_constants(float(gam[pid % H]))
            A_all[:, q * T : (q + 1) * T] = A.astype(np.float16)
            u_all[:, q] = u.astype(np.float16)
            gv_all[0, q * T : (q + 1) * T] = gv.astype(np.float16)
            GT_all[:, q * KB : (q + 1) * KB] = GT.astype(np.float16)
        in_maps.append(
            {
                "xv": xv,
                "g": gcol,
                "xp": xp,
                "A_all": A_all,
                "u_all": u_all,
                "gv_all": gv_all,
                "GT_all": GT_all,
            }
        )
    return in_maps


def kernel(tensor, gamma):
    from concourse.bass_utils import run_bass_kernel_spmd

    in_maps = _make_in_maps(tensor, gamma)
    nc = _build_program()
    res = run_bass_kernel_spmd(nc, in_maps, list(range(NCORES))).results
    y = np.empty((B * H, S, D), np.float32)
    for c in range(NCORES):
        yv = np.asarray(res[c]["yv"])  # [NV, D, S] fp16
        yp = np.asarray(res[c]["yp"])  # [NP, T, KB*D] fp16
        for v in range(NV):
            y[c * PAIRS + v] = yv[v].T
        for q in range(NP):
            y[c * PAIRS + NV + q] = (
                yp[q].reshape(T, KB, D).transpose(1, 0, 2).reshape(S, D)
            )
    return y.reshape(B, H, S, D)
